# revision 62
# baseline (speedup 1.0000x reference)
"""Masked-BCE mean loss kernel for Trainium2, data-parallel over 8 NeuronCores.

Math (targets t are exactly 0.0/1.0, C=2 label columns):
    bce(x, t) = softplus(x) - x*t = softplus(y),  y = (1-2t)*x = w*x
    row mask  = 1[t0 + t1 > 0]
    answer    = sum(mask * (bce0 + bce1)) / (B*C)

Input encoding (halves DMA -- the 8 MiB/core bf16 x+w pair was the HBM/ring
wall): t is stolen into the LSB of bf16 x ("LSB steal").  One 4 MiB tensor
ships per core; x loses its bottom mantissa bit (rel err 2^-8, unbiased
since t is independent of x).  On-device decode is all int16 bitwise ops at
full DVE speed:
    M   = (x' & 1) << 15        (tensor_scalar 2-op, 4x) - sign-flip masks
    Y   = x' ^ M                (tensor_tensor xor, 2x)  - exact w*x
    Vor = M0 | M1               (2x) - 0x8000 iff pair is unmasked
    Mp  = cast(Vor >> 15)       ({0,1} bf16 pair mask)

Work splits across chunks in two modes, balancing DVE against ACT with the
tensor engine absorbing the remainder:

  PAIR chunks (DVE-heavy, ACT-light):
    softplus(y0)+softplus(y1) = ln((1+E0)(1+E1)) = ln(1+r), r = E0+E1+E0*E1.
    Mp multiplies into r before the Ln (bias=1.0), so masked pairs give
    ln(1)=0 and the Ln's accum_out IS the masked partial sum; Ln runs on
    HALF the elements.

  FULL chunks (ACT-heavy, DVE-light, PE masks):
    S = ln(E+1) over all elements; PE accumulates the generalized diagonal
    of Mp[128-window]^T x [S0-win | S1-win] (FD=256) into PSUM -- the
    diagonal stripes hold sum(mask * (sp0+sp1)).

DMA discipline: one dma_start per chunk on nc.sync (HWDGE; every transfer
pays ~2us fixed and all FIFO on one ring set).  Engines run queues IN
ORDER, so emission is software-pipelined two chunks deep.  Exp+Ln pinned to
the single `natural_log_exp_and_others` table set (one ACT_TABLE_LOAD).

Host: answer = (ln accums + PSUM diag stripes) / (B*C) in f64.
"""

import sys

import numpy as np

for _p in ("/opt/trn_rl_repo",):
    if _p not in sys.path:
        sys.path.insert(0, _p)

import concourse.tile as tile  # noqa: E402
from concourse import bacc, mybir  # noqa: E402
from concourse.bass_utils import run_bass_kernel_spmd  # noqa: E402

N_CORES = 8
B = 8388608
C = 2
PAIRS = B // N_CORES  # 1048576 pairs per core
P = 128
COLS = 2 * PAIRS // P  # 16384 data columns per core
HCOLS = COLS // 2  # 8192 pair-columns per core

# chunk schedule: (h pair-cols, mode); FULL chunks first so the PE stripe
# matmuls drain while the PAIR chunks run; small head (fast start) and
# small tail (short serial drain)
CHUNKS = [
    (512, "full"),
    (1792, "full"),
    (1792, "full"),
    (1792, "pair"),
    (1792, "fullz"),
    (512, "pairacc"),
]
assert sum(h for h, _ in CHUNKS) == HCOLS
N_CHUNKS = len(CHUNKS)
TOTAL_MM = sum(h // 128 for h, m in CHUNKS if m in ("full", "pair", "pairsplit"))

dt = mybir.dt
AF = mybir.ActivationFunctionType
ALU = mybir.AluOpType

_CACHE: dict[str, object] = {}


def _patch_act_tables():
    """Pin Exp and Ln to the single covering table set (one ACT_TABLE_LOAD)."""
    if _CACHE.get("act_patched"):
        return
    import concourse.hw_specs as hw_specs

    orig = hw_specs.get_activation_tables

    def patched(module_arch):
        tabs = orig(module_arch)
        out = {}
        for name, funcs in tabs.items():
            if name == "natural_log_exp_and_others":
                out[name] = set(funcs)
            else:
                out[name] = set(funcs) - {AF.Exp, AF.Ln}
        return out

    bacc.get_activation_tables = patched
    _CACHE["act_patched"] = True


def _build_nc():
    _patch_act_tables()
    nc = bacc.Bacc(
        "TRN2", target_bir_lowering=False, debug=False, num_devices=N_CORES
    )
    xq_d = nc.dram_tensor("xq", [P, COLS], dt.bfloat16, kind="ExternalInput").ap()
    # one output tensor: cols 0..255 = PE stripe acc, 256.. = ln accums
    res_d = nc.dram_tensor(
        "res", [P, 256 + N_CHUNKS], dt.float32, kind="ExternalOutput"
    ).ap()

    with tile.TileContext(nc) as tc:
        with (
            tc.tile_pool(name="io", bufs=N_CHUNKS) as io_pool,
            tc.tile_pool(name="work", bufs=2) as work_pool,
            tc.tile_pool(name="ps", bufs=1, space="PSUM") as psum_pool,
            tc.tile_pool(name="outp", bufs=1) as out_pool,
        ):
            # tiny dummy Exp up front hoists the ~2.7us ACT_TABLE_LOAD off
            # the critical path (overlaps the first DMAs)
            warm = out_pool.tile([P, 8], dt.float32)
            nc.vector.memset(warm[:], 0.0)
            nc.scalar.activation(warm[:], warm[:], AF.Exp)

            res = out_pool.tile([P, 256 + N_CHUNKS], dt.float32)
            nc.vector.memset(res[:, 256:], 0.0)
            acc = psum_pool.tile([P, 256], dt.float32)

            # one input DMA per chunk; each chunk owns its tile
            XQ = []
            col0 = 0
            for ci, (h, _) in enumerate(CHUNKS):
                T = io_pool.tile([P, 2 * h], dt.bfloat16, tag="XQ",
                                 name=f"XQ{ci}")
                nc.sync.dma_start(T[:], xq_d[:, col0 : col0 + 2 * h])
                XQ.append(T)
                col0 += 2 * h

            E = [None] * N_CHUNKS
            Mp = [None] * N_CHUNKS
            mm_state = [0]

            def stage_B(ci):  # decode + exp
                h, mode = CHUNKS[ci]
                f = 2 * h
                Xi = XQ[ci][:].bitcast(dt.uint16)
                M = work_pool.tile([P, f], dt.uint16, tag="M")
                nc.vector.tensor_scalar(
                    M[:], Xi, 1, 15, ALU.bitwise_and, ALU.logical_shift_left
                )
                Yi = work_pool.tile([P, f], dt.uint16, tag="Yi")
                nc.vector.tensor_tensor(Yi[:], Xi, M[:], ALU.bitwise_xor)
                Vor = work_pool.tile([P, h], dt.uint16, tag="Vor")
                nc.vector.tensor_tensor(
                    Vor[:], M[:, :h], M[:, h:f], ALU.bitwise_or
                )
                src = Yi[:].bitcast(dt.bfloat16)
                if mode == "fullz":
                    # mask BEFORE exp: (Vor ^ 0x8000) >>arith 4 gives
                    # 0xF800 = -2^113 for masked pairs (no sign flips) and
                    # -0.0 for unmasked; z = y + F -> exp -> 0 for masked,
                    # so the full Ln's accum needs no Mp and no matmuls.
                    # min(Vor as int16, -2048): masked (0) -> -2048 = bits
                    # 0xF800 = bf16 -2^113; unmasked (-32768 = 0x8000) ->
                    # unchanged = bf16 -0.0
                    F = work_pool.tile([P, h], dt.int16, tag="F")
                    nc.vector.tensor_scalar(
                        F[:], Vor[:].bitcast(dt.int16), -2048, None, ALU.min
                    )
                    Fb = F[:].bitcast(dt.bfloat16)
                    Z = work_pool.tile([P, f], dt.bfloat16, tag="Z")
                    nc.vector.tensor_tensor(
                        Z[:, :h], src[:, :h], Fb, ALU.add
                    )
                    nc.vector.tensor_tensor(
                        Z[:, h:f], src[:, h:f], Fb, ALU.add
                    )
                    src = Z[:]
                else:
                    # 0x8000 >> 1 = 0x4000 = bf16 2.0 -> pair mask in
                    # {0, 2.0}; the factor 2 divides out on the host
                    Mpi = work_pool.tile(
                        [P, h], dt.uint16, tag="Mpi", name=f"Mpi{ci}", bufs=3
                    )
                    nc.vector.tensor_scalar(
                        Mpi[:], Vor[:], 1, None, ALU.logical_shift_right
                    )
                    Mp[ci] = Mpi[:].bitcast(dt.bfloat16)
                E[ci] = work_pool.tile(
                    [P, f], dt.bfloat16, tag="E", name=f"E{ci}", bufs=3
                )
                nc.scalar.activation(E[ci][:], src, AF.Exp)

            def stage_C(ci):
                h, mode = CHUNKS[ci]
                f = 2 * h
                Ec = E[ci]
                if mode == "fullz":
                    # pre-masked: the full Ln's accum IS the masked sum
                    L = work_pool.tile([P, f], dt.bfloat16, tag="S")
                    nc.scalar.activation(
                        L[:], Ec[:], AF.Ln, bias=1.0,
                        accum_out=res[:, 256 + ci : 257 + ci],
                    )
                    return
                if mode.startswith("pair"):
                    P2 = work_pool.tile([P, h], dt.bfloat16, tag="P2")
                    nc.vector.scalar_tensor_tensor(
                        P2[:], Ec[:, :h], 1.0, Ec[:, h:f], ALU.add, ALU.mult
                    )
                    r = work_pool.tile([P, h], dt.bfloat16, tag="r")
                    nc.vector.tensor_tensor(r[:], P2[:], Ec[:, :h], ALU.add)
                    if mode == "pairacc":
                        # mask on DVE + fused ln accum -> no trailing matmuls
                        # (used for the last chunk: short drain).  Mp is
                        # {0, 2.0}; scale=0.5 restores ln(1 + mask*r).
                        rm = work_pool.tile([P, h], dt.bfloat16, tag="rm")
                        nc.vector.tensor_tensor(
                            rm[:], r[:], Mp[ci], ALU.mult
                        )
                        L = work_pool.tile([P, h], dt.bfloat16, tag="L")
                        nc.scalar.activation(
                            L[:], rm[:], AF.Ln, bias=1.0, scale=0.5,
                            accum_out=res[:, 256 + ci : 257 + ci],
                        )
                        return
                    # unmasked pair softplus sums; PE applies the mask.
                    # pairsplit: ln in two halves so the first matmul batch
                    # overlaps the second ln (shortens the drain of the
                    # last PE-bearing chunk)
                    L = work_pool.tile([P, h], dt.bfloat16, tag="L")
                    parts = (
                        [(0, h // 2), (h // 2, h)]
                        if mode == "pairsplit"
                        else [(0, h)]
                    )
                    for lo, hi in parts:
                        nc.scalar.activation(
                            L[:, lo:hi], r[:, lo:hi], AF.Ln, bias=1.0
                        )
                        for c in range(lo, hi, 128):
                            nc.tensor.matmul(
                                acc[:, :128],
                                lhsT=Mp[ci][:, c : c + 128],  # {0,2}
                                rhs=L[:, c : c + 128],
                                start=(mm_state[0] == 0),
                                stop=(mm_state[0] == TOTAL_MM - 1),
                            )
                            mm_state[0] += 1
                else:
                    S = work_pool.tile([P, f], dt.bfloat16, tag="S")
                    nc.scalar.activation(S[:], Ec[:], AF.Ln, bias=1.0)
                    Sv = S[:].rearrange("p (b h) -> p b h", b=2)
                    for c in range(0, h, 128):
                        nc.tensor.matmul(
                            acc[:],
                            lhsT=Mp[ci][:, c : c + 128],  # {0,2}: /2 on host
                            rhs=Sv[:, :, c : c + 128],
                            start=(mm_state[0] == 0),
                            stop=(mm_state[0] == TOTAL_MM - 1),
                        )
                        mm_state[0] += 1

            # software-pipelined emission, two chunks deep
            stage_B(0)
            stage_B(1)
            for ci in range(2, N_CHUNKS):
                stage_B(ci)
                stage_C(ci - 2)
            stage_C(N_CHUNKS - 2)
            # the last chunk (pairacc) issues no matmuls, so the PSUM
            # stripe copy-out and its 128KB DMA overlap it; only the tiny
            # accum-column DMA remains on the drain path
            nc.vector.tensor_copy(res[:, :256], acc[:])
            nc.sync.dma_start(res_d[:, :256], res[:, :256])
            stage_C(N_CHUNKS - 1)
            nc.sync.dma_start(res_d[:, 256:], res[:, 256:])

    nc.compile()
    return nc


def _get_nc():
    if "nc" not in _CACHE:
        _CACHE["nc"] = _build_nc()
    return _CACHE["nc"]


def _reduce_outputs(results: list[np.ndarray]) -> np.ndarray:
    j = np.arange(P)
    total = 0.0
    for re_ in results:
        a64 = re_.astype(np.float64)
        total += a64[:, 256:].sum()  # pair chunks: ln accums
        # full chunks: stripes carry the {0,2} mask -> halve
        total += 0.5 * (a64[j, j].sum() + a64[j, 128 + j].sum())
    return np.asarray(total / (B * C), dtype=np.float32)


def make_in_maps(inputs: np.ndarray, targets: np.ndarray) -> list[dict]:
    import ml_dtypes

    # Per core, chunk-major [col0-block | col1-block] halves so every DVE op
    # is unit-stride (2x mode); t stolen into the LSB of bf16 x.
    x = np.ascontiguousarray(inputs, dtype=np.float32).reshape(
        N_CORES, PAIRS, C
    )
    t = np.ascontiguousarray(targets, dtype=np.float32).reshape(
        N_CORES, PAIRS, C
    )
    xp = x.transpose(0, 2, 1).reshape(N_CORES, C, P, HCOLS)
    tp = t.transpose(0, 2, 1).reshape(N_CORES, C, P, HCOLS)

    xq = np.empty((N_CORES, P, COLS), dtype=np.uint16)
    col0 = 0
    off = 0
    for h, _ in CHUNKS:
        for c in range(C):
            xb = xp[:, c, :, off : off + h].astype(ml_dtypes.bfloat16)
            tb = tp[:, c, :, off : off + h] != 0.0
            xq[:, :, col0 : col0 + h] = (
                xb.view(np.uint16) & np.uint16(0xFFFE)
            ) | tb.astype(np.uint16)
            col0 += h
        off += h
    return [
        {"xq": xq[c].view(ml_dtypes.bfloat16)} for c in range(N_CORES)
    ]


def kernel(inputs: np.ndarray, targets: np.ndarray) -> np.ndarray:
    nc = _get_nc()
    in_maps = make_in_maps(inputs, targets)
    res = run_bass_kernel_spmd(nc, in_maps, list(range(N_CORES)))
    return _reduce_outputs(
        [res.results[c]["res"] for c in range(N_CORES)]
    )


# revision 64
# speedup vs baseline: 1.0377x; 1.0377x over previous
"""Masked-BCE mean loss kernel for Trainium2, data-parallel over 8 NeuronCores.

Math (targets t are exactly 0.0/1.0, C=2 label columns):
    bce(x, t) = softplus(x) - x*t = softplus(y),  y = (1-2t)*x = w*x
    row mask  = 1[t0 + t1 > 0]
    answer    = sum(mask * (bce0 + bce1)) / (B*C)

Input encoding (halves DMA -- the 8 MiB/core bf16 x+w pair was the HBM/ring
wall): t is stolen into the LSB of bf16 x ("LSB steal").  One 4 MiB tensor
ships per core; x loses its bottom mantissa bit (rel err 2^-8, unbiased
since t is independent of x).  On-device decode is all int16 bitwise ops at
full DVE speed:
    M   = (x' & 1) << 15        (tensor_scalar 2-op, 4x) - sign-flip masks
    Y   = x' ^ M                (tensor_tensor xor, 2x)  - exact w*x
    Vor = M0 | M1               (2x) - 0x8000 iff pair is unmasked
    Mp  = cast(Vor >> 15)       ({0,1} bf16 pair mask)

Work splits across chunks in two modes, balancing DVE against ACT with the
tensor engine absorbing the remainder:

  PAIR chunks (DVE-heavy, ACT-light):
    softplus(y0)+softplus(y1) = ln((1+E0)(1+E1)) = ln(1+r), r = E0+E1+E0*E1.
    Mp multiplies into r before the Ln (bias=1.0), so masked pairs give
    ln(1)=0 and the Ln's accum_out IS the masked partial sum; Ln runs on
    HALF the elements.

  FULL chunks (ACT-heavy, DVE-light, PE masks):
    S = ln(E+1) over all elements; PE accumulates the generalized diagonal
    of Mp[128-window]^T x [S0-win | S1-win] (FD=256) into PSUM -- the
    diagonal stripes hold sum(mask * (sp0+sp1)).

DMA discipline: one dma_start per chunk on nc.sync (HWDGE; every transfer
pays ~2us fixed and all FIFO on one ring set).  Engines run queues IN
ORDER, so emission is software-pipelined two chunks deep.  Exp+Ln pinned to
the single `natural_log_exp_and_others` table set (one ACT_TABLE_LOAD).

Host: answer = (ln accums + PSUM diag stripes) / (B*C) in f64.
"""

import sys

import numpy as np

for _p in ("/opt/trn_rl_repo",):
    if _p not in sys.path:
        sys.path.insert(0, _p)

import concourse.tile as tile  # noqa: E402
from concourse import bacc, mybir  # noqa: E402
from concourse.bass_utils import run_bass_kernel_spmd  # noqa: E402

N_CORES = 8
B = 8388608
C = 2
PAIRS = B // N_CORES  # 1048576 pairs per core
P = 128
COLS = 2 * PAIRS // P  # 16384 data columns per core
HCOLS = COLS // 2  # 8192 pair-columns per core

# chunk schedule: (h pair-cols, mode); FULL chunks first so the PE stripe
# matmuls drain while the PAIR chunks run; small head (fast start) and
# small tail (short serial drain)
CHUNKS = [
    (512, "full"),
    (1792, "full"),
    (1792, "full"),
    (1792, "pair"),
    (1792, "pairsplit"),
    (512, "pairacc"),
]
assert sum(h for h, _ in CHUNKS) == HCOLS
N_CHUNKS = len(CHUNKS)
TOTAL_MM = sum(h // 128 for h, m in CHUNKS if m != "pairacc")

dt = mybir.dt
AF = mybir.ActivationFunctionType
ALU = mybir.AluOpType

_CACHE: dict[str, object] = {}


def _patch_act_tables():
    """Pin Exp and Ln to the single covering table set (one ACT_TABLE_LOAD)."""
    if _CACHE.get("act_patched"):
        return
    import concourse.hw_specs as hw_specs

    orig = hw_specs.get_activation_tables

    def patched(module_arch):
        tabs = orig(module_arch)
        out = {}
        for name, funcs in tabs.items():
            if name == "natural_log_exp_and_others":
                out[name] = set(funcs)
            else:
                out[name] = set(funcs) - {AF.Exp, AF.Ln}
        return out

    bacc.get_activation_tables = patched
    _CACHE["act_patched"] = True


def _build_nc():
    _patch_act_tables()
    nc = bacc.Bacc(
        "TRN2", target_bir_lowering=False, debug=False, num_devices=N_CORES
    )
    xq_d = nc.dram_tensor("xq", [P, COLS], dt.bfloat16, kind="ExternalInput").ap()
    # one output tensor: cols 0..255 = PE stripe acc, 256.. = ln accums
    res_d = nc.dram_tensor(
        "res", [P, 256 + N_CHUNKS], dt.float32, kind="ExternalOutput"
    ).ap()

    with tile.TileContext(nc) as tc:
        with (
            tc.tile_pool(name="io", bufs=N_CHUNKS) as io_pool,
            tc.tile_pool(name="work", bufs=2) as work_pool,
            tc.tile_pool(name="ps", bufs=1, space="PSUM") as psum_pool,
            tc.tile_pool(name="outp", bufs=1) as out_pool,
        ):
            # tiny dummy Exp up front hoists the ~2.7us ACT_TABLE_LOAD off
            # the critical path (overlaps the first DMAs)
            warm = out_pool.tile([P, 8], dt.float32)
            nc.vector.memset(warm[:], 0.0)
            nc.scalar.activation(warm[:], warm[:], AF.Exp)

            res = out_pool.tile([P, 256 + N_CHUNKS], dt.float32)
            nc.vector.memset(res[:, 256:], 0.0)
            acc = psum_pool.tile([P, 256], dt.float32)

            # one input DMA per chunk; each chunk owns its tile
            XQ = []
            col0 = 0
            for ci, (h, _) in enumerate(CHUNKS):
                T = io_pool.tile([P, 2 * h], dt.bfloat16, tag="XQ",
                                 name=f"XQ{ci}")
                nc.sync.dma_start(T[:], xq_d[:, col0 : col0 + 2 * h])
                XQ.append(T)
                col0 += 2 * h

            E = [None] * N_CHUNKS
            Mp = [None] * N_CHUNKS
            mm_state = [0]

            def stage_B(ci):  # decode + exp
                h, _ = CHUNKS[ci]
                f = 2 * h
                Xi = XQ[ci][:].bitcast(dt.uint16)
                M = work_pool.tile([P, f], dt.uint16, tag="M")
                nc.vector.tensor_scalar(
                    M[:], Xi, 1, 15, ALU.bitwise_and, ALU.logical_shift_left
                )
                Yi = work_pool.tile([P, f], dt.uint16, tag="Yi")
                nc.vector.tensor_tensor(Yi[:], Xi, M[:], ALU.bitwise_xor)
                E[ci] = work_pool.tile(
                    [P, f], dt.bfloat16, tag="E", name=f"E{ci}", bufs=3
                )
                nc.scalar.activation(
                    E[ci][:], Yi[:].bitcast(dt.bfloat16), AF.Exp
                )
                Vor = work_pool.tile([P, h], dt.uint16, tag="Vor")
                nc.vector.tensor_tensor(
                    Vor[:], M[:, :h], M[:, h:f], ALU.bitwise_or
                )
                # 0x8000 >> 1 = 0x4000 = bf16 2.0 -> pair mask in {0, 2.0};
                # the factor 2 divides out on the host
                Mpi = work_pool.tile(
                    [P, h], dt.uint16, tag="Mpi", name=f"Mpi{ci}", bufs=3
                )
                nc.vector.tensor_scalar(
                    Mpi[:], Vor[:], 1, None, ALU.logical_shift_right
                )
                Mp[ci] = Mpi[:].bitcast(dt.bfloat16)

            def stage_C(ci):
                h, mode = CHUNKS[ci]
                f = 2 * h
                Ec = E[ci]
                if mode.startswith("pair"):
                    P2 = work_pool.tile([P, h], dt.bfloat16, tag="P2")
                    nc.vector.scalar_tensor_tensor(
                        P2[:], Ec[:, :h], 1.0, Ec[:, h:f], ALU.add, ALU.mult
                    )
                    r = work_pool.tile([P, h], dt.bfloat16, tag="r")
                    nc.vector.tensor_tensor(r[:], P2[:], Ec[:, :h], ALU.add)
                    if mode == "pairacc":
                        # mask on DVE + fused ln accum -> no trailing matmuls
                        # (used for the last chunk: short drain).  Mp is
                        # {0, 2.0}; scale=0.5 restores ln(1 + mask*r).
                        rm = work_pool.tile([P, h], dt.bfloat16, tag="rm")
                        nc.vector.tensor_tensor(
                            rm[:], r[:], Mp[ci], ALU.mult
                        )
                        L = work_pool.tile([P, h], dt.bfloat16, tag="L")
                        nc.scalar.activation(
                            L[:], rm[:], AF.Ln, bias=1.0, scale=0.5,
                            accum_out=res[:, 256 + ci : 257 + ci],
                        )
                        return
                    # unmasked pair softplus sums; PE applies the mask.
                    # pairsplit: ln in two halves so the first matmul batch
                    # overlaps the second ln (shortens the drain of the
                    # last PE-bearing chunk)
                    L = work_pool.tile([P, h], dt.bfloat16, tag="L")
                    # asymmetric split: small first Ln starts the serial
                    # matmul chain sooner; the longer second Ln hides under it
                    parts = (
                        [(0, 512), (512, h)]
                        if mode == "pairsplit"
                        else [(0, h)]
                    )
                    for lo, hi in parts:
                        nc.scalar.activation(
                            L[:, lo:hi], r[:, lo:hi], AF.Ln, bias=1.0
                        )
                        for c in range(lo, hi, 128):
                            nc.tensor.matmul(
                                acc[:, :128],
                                lhsT=Mp[ci][:, c : c + 128],  # {0,2}
                                rhs=L[:, c : c + 128],
                                start=(mm_state[0] == 0),
                                stop=(mm_state[0] == TOTAL_MM - 1),
                            )
                            mm_state[0] += 1
                else:
                    S = work_pool.tile([P, f], dt.bfloat16, tag="S")
                    nc.scalar.activation(S[:], Ec[:], AF.Ln, bias=1.0)
                    Sv = S[:].rearrange("p (b h) -> p b h", b=2)
                    for c in range(0, h, 128):
                        nc.tensor.matmul(
                            acc[:],
                            lhsT=Mp[ci][:, c : c + 128],  # {0,2}: /2 on host
                            rhs=Sv[:, :, c : c + 128],
                            start=(mm_state[0] == 0),
                            stop=(mm_state[0] == TOTAL_MM - 1),
                        )
                        mm_state[0] += 1

            # software-pipelined emission, two chunks deep
            stage_B(0)
            stage_B(1)
            for ci in range(2, N_CHUNKS):
                stage_B(ci)
                stage_C(ci - 2)
            stage_C(N_CHUNKS - 2)
            # the last chunk (pairacc) issues no matmuls, so the PSUM
            # stripe copy-out and its 128KB DMA overlap it; only the tiny
            # accum-column DMA remains on the drain path
            nc.vector.tensor_copy(res[:, :256], acc[:])
            nc.sync.dma_start(res_d[:, :256], res[:, :256])
            stage_C(N_CHUNKS - 1)
            nc.sync.dma_start(res_d[:, 256:], res[:, 256:])

    nc.compile()
    return nc


def _get_nc():
    if "nc" not in _CACHE:
        _CACHE["nc"] = _build_nc()
    return _CACHE["nc"]


def _reduce_outputs(results: list[np.ndarray]) -> np.ndarray:
    j = np.arange(P)
    total = 0.0
    for re_ in results:
        a64 = re_.astype(np.float64)
        total += a64[:, 256:].sum()  # pair chunks: ln accums
        # full chunks: stripes carry the {0,2} mask -> halve
        total += 0.5 * (a64[j, j].sum() + a64[j, 128 + j].sum())
    return np.asarray(total / (B * C), dtype=np.float32)


def make_in_maps(inputs: np.ndarray, targets: np.ndarray) -> list[dict]:
    import ml_dtypes

    # Per core, chunk-major [col0-block | col1-block] halves so every DVE op
    # is unit-stride (2x mode); t stolen into the LSB of bf16 x.
    x = np.ascontiguousarray(inputs, dtype=np.float32).reshape(
        N_CORES, PAIRS, C
    )
    t = np.ascontiguousarray(targets, dtype=np.float32).reshape(
        N_CORES, PAIRS, C
    )
    xp = x.transpose(0, 2, 1).reshape(N_CORES, C, P, HCOLS)
    tp = t.transpose(0, 2, 1).reshape(N_CORES, C, P, HCOLS)

    xq = np.empty((N_CORES, P, COLS), dtype=np.uint16)
    col0 = 0
    off = 0
    for h, _ in CHUNKS:
        for c in range(C):
            xb = xp[:, c, :, off : off + h].astype(ml_dtypes.bfloat16)
            tb = tp[:, c, :, off : off + h] != 0.0
            xq[:, :, col0 : col0 + h] = (
                xb.view(np.uint16) & np.uint16(0xFFFE)
            ) | tb.astype(np.uint16)
            col0 += h
        off += h
    return [
        {"xq": xq[c].view(ml_dtypes.bfloat16)} for c in range(N_CORES)
    ]


def kernel(inputs: np.ndarray, targets: np.ndarray) -> np.ndarray:
    nc = _get_nc()
    in_maps = make_in_maps(inputs, targets)
    res = run_bass_kernel_spmd(nc, in_maps, list(range(N_CORES)))
    return _reduce_outputs(
        [res.results[c]["res"] for c in range(N_CORES)]
    )
